# revision 5
# baseline (speedup 1.0000x reference)
"""Trainium2 Bass kernel for nn_Decoder_74234214744419 (moe_routing).

Decoder layer: embedding lookup -> LN1 -> RoPE causal attention ->
out_proj+residual -> LN2 -> router (top-2 of 8) -> 8 dense SwiGLU experts ->
weighted combine; plus load-balance aux loss.

Distribution over 8 NeuronCores:
 - attention head-parallel (16 heads -> 2 per core); QKV/attention math in
   fp32 (the top-2 routing is a discrete function of the logits, which sit
   downstream of attention - bf16 noise there flips expert selections vs the
   reference, so this path stays fp32)
 - out_proj computed as per-core partials over its own heads, combined with
   an on-device ReduceScatter -> each core owns a 512-token slice
 - LN2 + router token-sharded; router weights AllGather'd
 - nm^T (bf16) AllGather'd; experts run expert-parallel (core c = expert c),
   MoE matmuls in bf16
 - weighted expert outputs combined with an on-device ReduceScatter
 - host: embedding gather (pure data movement), aux loss from
   device-computed logits, final slice concat

RoPE trick: the interleaved (even,odd) pair rotation is turned into a
contiguous-half rotation by permuting the q/k weight COLUMNS on the host
(per-head de-interleave). Scores are invariant to any per-head permutation
applied to both q and k.
"""
import sys
if '/opt/trn_rl_repo' not in sys.path:
    sys.path.insert(0, '/opt/trn_rl_repo')

import numpy as np
import ml_dtypes

import concourse.bass as bass
import concourse.tile as tile
from concourse import bacc, mybir
from concourse.bass_utils import run_bass_kernel_spmd
from concourse.masks import make_identity, make_causal_mask

F32 = mybir.dt.float32
BF16 = mybir.dt.bfloat16

B, N, D, H, DH, E, HID, V = 2, 2048, 1024, 16, 64, 8, 4096, 32000
NTOK = B * N            # 4096
NC = 8                  # cores
TPC = NTOK // NC        # 512 tokens per core
NT = NTOK // 128        # 32 token tiles
KK = D // 128           # 8 contraction tiles over D
KM = HID // 128         # 32 contraction tiles over HID
NEG = -400.0            # additive causal mask; Exp applies scale=0.125 -> exp(-50)

_CACHE = {}


def _build(debug=False):
    key = ('nc', debug)
    if key in _CACHE:
        return _CACHE[key]
    nc = bacc.Bacc(num_devices=NC)

    # ---- I/O ----
    h_in = nc.dram_tensor("h", [NTOK, D], F32, kind="ExternalInput")
    h_sl = nc.dram_tensor("h_sl", [TPC, D], F32, kind="ExternalInput")
    wqkv = nc.dram_tensor("wqkv", [D, 384], F32, kind="ExternalInput")
    outp = nc.dram_tensor("outp", [128, D], F32, kind="ExternalInput")
    ln1w = nc.dram_tensor("ln1w", [D], F32, kind="ExternalInput")
    ln1b = nc.dram_tensor("ln1b", [D], F32, kind="ExternalInput")
    ln2w = nc.dram_tensor("ln2w", [D], F32, kind="ExternalInput")
    ln2b = nc.dram_tensor("ln2b", [D], F32, kind="ExternalInput")
    gw = nc.dram_tensor("gw", [D, E], F32, kind="ExternalInput")
    gb = nc.dram_tensor("gb", [E], F32, kind="ExternalInput")
    w1e = nc.dram_tensor("w1e", [D, HID], BF16, kind="ExternalInput")
    w2e = nc.dram_tensor("w2e", [D, HID], BF16, kind="ExternalInput")
    w3e = nc.dram_tensor("w3e", [HID, D], BF16, kind="ExternalInput")
    b1e = nc.dram_tensor("b1e", [HID], F32, kind="ExternalInput")
    b2e = nc.dram_tensor("b2e", [HID], F32, kind="ExternalInput")
    b3e = nc.dram_tensor("b3e", [D], F32, kind="ExternalInput")
    cos_in = nc.dram_tensor("cos", [NTOK, 32], F32, kind="ExternalInput")
    sin_in = nc.dram_tensor("sin", [NTOK, 32], F32, kind="ExternalInput")
    onehot = nc.dram_tensor("onehot", [E], F32, kind="ExternalInput")

    out_sl = nc.dram_tensor("out_sl", [TPC, D], F32, kind="ExternalOutput")
    logits_sl = nc.dram_tensor("logits_sl", [TPC, E], F32, kind="ExternalOutput")
    dbg = {}
    if debug:
        dbg['qkT0'] = nc.dram_tensor("dbg_qkT0", [128, NTOK], F32, kind="ExternalOutput")
        dbg['qkT1'] = nc.dram_tensor("dbg_qkT1", [128, NTOK], F32, kind="ExternalOutput")
        dbg['attT'] = nc.dram_tensor("dbg_attT", [128, NTOK], F32, kind="ExternalOutput")
        dbg['nm'] = nc.dram_tensor("dbg_nm", [TPC, D], F32, kind="ExternalOutput")
        dbg['w'] = nc.dram_tensor("dbg_w", [TPC, E], F32, kind="ExternalOutput")
        dbg['moein'] = nc.dram_tensor("dbg_moein", [NTOK, D], F32, kind="ExternalOutput")

    # ---- internal DRAM (collectives) ----
    rs1_in = nc.dram_tensor("rs1_in", [NTOK, D], F32)
    rs1_out = nc.dram_tensor("rs1_out", [TPC, D], F32)
    nmT_in = nc.dram_tensor("nmT_in", [D, TPC], BF16)
    nmT_full = nc.dram_tensor("nmT_full", [NC * D, TPC], BF16, addr_space="Shared")
    w_in = nc.dram_tensor("w_in", [TPC, E], F32)
    w_full = nc.dram_tensor("w_full", [NTOK, E], F32, addr_space="Shared")
    rs2_in = nc.dram_tensor("rs2_in", [NTOK, D], F32)
    rs2_out = nc.dram_tensor("rs2_out", [TPC, D], F32)
    RG = [list(range(NC))]

    with tile.TileContext(nc) as tc:
        with tc.tile_pool(name="const", bufs=1) as const:
            ident = const.tile([128, 128], F32)
            make_identity(nc, ident)
            cmask = const.tile([128, 128], F32)
            make_causal_mask(nc, cmask, mask_val=NEG)
            eps_t = const.tile([128, 1], F32)
            nc.vector.memset(eps_t, 1e-5)

            def bcast_load(vec, n):
                t = const.tile([128, n], F32, tag=f"bc_{vec.name}")
                src = bass.AP(tensor=vec, offset=0, ap=[[0, 128], [1, n]])
                nc.sync.dma_start(out=t[:], in_=src)
                return t

            ln1w_bc = bcast_load(ln1w, D)
            ln1b_bc = bcast_load(ln1b, D)
            ln2w_bc = bcast_load(ln2w, D)
            ln2b_bc = bcast_load(ln2b, D)
            b3_bc = bcast_load(b3e, D)
            gb_bc = bcast_load(gb, E)
            oh_bc = bcast_load(onehot, E)

            wqkv_t = const.tile([128, KK, 384], F32)
            nc.sync.dma_start(out=wqkv_t[:],
                              in_=wqkv.ap().rearrange("(k p) c -> p k c", p=128))
            outp_t = const.tile([128, D], F32)
            nc.sync.dma_start(out=outp_t[:], in_=outp[:, :])
            gw_t = const.tile([128, KK, E], F32)
            nc.sync.dma_start(out=gw_t[:],
                              in_=gw.ap().rearrange("(k p) e -> p k e", p=128))
            # b1/b2 partition-major: element m*128+p -> [p, m]
            b1_t = const.tile([128, KM], F32)
            nc.sync.dma_start(out=b1_t[:],
                              in_=bass.AP(tensor=b1e, offset=0, ap=[[1, 128], [128, KM]]))
            b2_t = const.tile([128, KM], F32)
            nc.sync.dma_start(out=b2_t[:],
                              in_=bass.AP(tensor=b2e, offset=0, ap=[[1, 128], [128, KM]]))

            # persistent activation tensors (live through phases 1-3 only)
            persist_cm = tc.tile_pool(name="persist", bufs=1)
            persist = persist_cm.__enter__()
            qkT0 = persist.tile([128, NTOK], F32, tag="qkT0")  # q^T (2 heads)
            qkT1 = persist.tile([128, NTOK], F32, tag="qkT1")  # k^T
            v_t = persist.tile([128, NT, 128], F32, tag="v")   # v token-major
            attT = persist.tile([128, NTOK], F32, tag="attT")  # att^T (2 heads)

            # ================= Phase 1+2: LN1 + QKV + RoPE =================
            with tc.tile_pool(name="p12", bufs=3) as p12, \
                 tc.tile_pool(name="p12ps", bufs=2, space="PSUM") as p12ps:
                for t in range(NT):
                    h_t = p12.tile([128, D], F32, tag="h")
                    nc.sync.dma_start(out=h_t[:], in_=h_in[128 * t:128 * (t + 1), :])
                    # LN1
                    nst = p12.tile([128, 2, 6], F32, tag="nst")
                    hg = h_t[:].rearrange("p (g d) -> p g d", g=2)
                    for g in range(2):
                        nc.vector.bn_stats(out=nst[:, g, :], in_=hg[:, g, :])
                    mv = p12.tile([128, 2], F32, tag="mv")
                    nc.vector.bn_aggr(out=mv[:], in_=nst[:])
                    sd = p12.tile([128, 1], F32, tag="sd")
                    nc.scalar.activation(sd[:], mv[:, 1:2],
                                         mybir.ActivationFunctionType.Sqrt,
                                         bias=eps_t[:])
                    rstd = p12.tile([128, 1], F32, tag="rstd")
                    nc.vector.reciprocal(rstd[:], sd[:])
                    pn_t = p12.tile([128, D], F32, tag="pn")
                    nc.vector.tensor_scalar(out=pn_t[:], in0=h_t[:],
                                            scalar1=mv[:, 0:1], scalar2=rstd[:],
                                            op0=mybir.AluOpType.subtract,
                                            op1=mybir.AluOpType.mult)
                    nc.vector.tensor_mul(pn_t[:], pn_t[:], ln1w_bc[:])
                    nc.vector.tensor_add(pn_t[:], pn_t[:], ln1b_bc[:])
                    # transpose pn tile -> 8 [128,128] slices (D on partitions)
                    trs = p12.tile([128, KK, 128], F32, tag="trs")
                    for k in range(KK):
                        trp = p12ps.tile([128, 128], F32, tag="trp")
                        nc.tensor.transpose(trp[:], pn_t[:, 128 * k:128 * (k + 1)],
                                            ident[:])
                        nc.vector.tensor_copy(trs[:, k, :], trp[:])
                    # qkv = pn @ wqkv : [128 tok, 384]
                    qkv_ps = p12ps.tile([128, 384], F32, tag="qkv")
                    for k in range(KK):
                        nc.tensor.matmul(qkv_ps[:], trs[:, k, :], wqkv_t[:, k, :],
                                         start=(k == 0), stop=(k == KK - 1))
                    # rope on q/k blocks (per-head [e(32) o(32)] layout)
                    cs = p12.tile([128, 32], F32, tag="cs")
                    nc.sync.dma_start(out=cs[:], in_=cos_in[128 * t:128 * (t + 1), :])
                    sn = p12.tile([128, 32], F32, tag="sn")
                    nc.sync.dma_start(out=sn[:], in_=sin_in[128 * t:128 * (t + 1), :])
                    rot = p12.tile([128, 256], F32, tag="rot")
                    t1 = p12.tile([128, 32], F32, tag="t1")
                    t2 = p12.tile([128, 32], F32, tag="t2")
                    for qk in range(2):
                        for th in range(2):
                            base = 128 * qk + 64 * th
                            qe = qkv_ps[:, base:base + 32]
                            qo = qkv_ps[:, base + 32:base + 64]
                            re = rot[:, base:base + 32]
                            ro = rot[:, base + 32:base + 64]
                            nc.vector.tensor_mul(t1[:], qe, cs[:])
                            nc.vector.tensor_mul(t2[:], qo, sn[:])
                            nc.vector.tensor_sub(re, t1[:], t2[:])
                            nc.vector.tensor_mul(t1[:], qe, sn[:])
                            nc.vector.tensor_mul(t2[:], qo, cs[:])
                            nc.vector.tensor_add(ro, t1[:], t2[:])
                    # v
                    nc.vector.tensor_copy(v_t[:, t, :], qkv_ps[:, 256:384])
                    # transpose rot -> qkT0/qkT1 columns
                    for blk, dst in ((0, qkT0), (1, qkT1)):
                        trp2 = p12ps.tile([128, 128], F32, tag="trp")
                        nc.tensor.transpose(trp2[:],
                                            rot[:, 128 * blk:128 * (blk + 1)],
                                            ident[:])
                        nc.vector.tensor_copy(dst[:, 128 * t:128 * (t + 1)], trp2[:])

            if debug:
                nc.sync.dma_start(out=dbg['qkT0'][:, :], in_=qkT0[:])
                nc.sync.dma_start(out=dbg['qkT1'][:, :], in_=qkT1[:])

            # ================= Phase A: causal attention =================
            with tc.tile_pool(name="pat", bufs=2) as pat, \
                 tc.tile_pool(name="patp", bufs=2, space="PSUM") as patp:
                for b in range(B):
                    for hh in range(2):
                        hrow = 64 * hh
                        for qt in range(16):
                            qoff = 2048 * b + 128 * qt
                            nkey = 128 * (qt + 1)
                            prow = pat.tile([128, 2048], F32, tag="prow")
                            ltot = pat.tile([128, 1], F32, tag="ltot")
                            # score chunks of <=512 keys
                            koff = 0
                            ci = 0
                            while koff < nkey:
                                csz = min(512, nkey - koff)
                                sps = patp.tile([128, 512], F32, tag="sps")
                                nc.tensor.matmul(
                                    sps[:, :csz],
                                    qkT0[hrow:hrow + 64, qoff:qoff + 128],
                                    qkT1[hrow:hrow + 64,
                                         2048 * b + koff:2048 * b + koff + csz],
                                    start=True, stop=True)
                                if koff + csz == nkey:  # diagonal block
                                    nc.vector.tensor_add(sps[:, csz - 128:csz],
                                                         sps[:, csz - 128:csz],
                                                         cmask[:])
                                lsum = pat.tile([128, 1], F32, tag="lsum")
                                nc.scalar.activation(prow[:, koff:koff + csz],
                                                     sps[:, :csz],
                                                     mybir.ActivationFunctionType.Exp,
                                                     scale=0.125, accum_out=lsum[:])
                                if ci == 0:
                                    nc.vector.tensor_copy(ltot[:], lsum[:])
                                else:
                                    nc.vector.tensor_add(ltot[:], ltot[:], lsum[:])
                                koff += csz
                                ci += 1
                            linv = pat.tile([128, 1], F32, tag="linv")
                            nc.vector.reciprocal(linv[:], ltot[:])
                            nc.vector.tensor_scalar_mul(prow[:, :nkey],
                                                        prow[:, :nkey], linv[:])
                            # P^T blocks + PV accumulation
                            attps = patp.tile([64, 128], F32, tag="attps")
                            for j in range(qt + 1):
                                ptp = patp.tile([128, 128], F32, tag="ptp")
                                nc.tensor.transpose(
                                    ptp[:], prow[:, 128 * j:128 * (j + 1)], ident[:])
                                pts = pat.tile([128, 128], F32, tag="pts")
                                nc.vector.tensor_copy(pts[:], ptp[:])
                                nc.tensor.matmul(
                                    attps[:], v_t[:, 16 * b + j, hrow:hrow + 64],
                                    pts[:], start=(j == 0), stop=(j == qt),
                                    skip_group_check=True)
                            nc.vector.tensor_copy(attT[hrow:hrow + 64,
                                                       qoff:qoff + 128], attps[:])

            if debug:
                nc.sync.dma_start(out=dbg['attT'][:, :], in_=attT[:])

            # ============ Phase 3: out_proj partials + RS + LN2 + router ============
            with tc.tile_pool(name="p3", bufs=2) as p3, \
                 tc.tile_pool(name="p3ps", bufs=2, space="PSUM") as p3ps:
                for t in range(NT):
                    po = p3.tile([128, D], F32, tag="po")
                    for nh in range(2):
                        pops = p3ps.tile([128, 512], F32, tag="pops")
                        nc.tensor.matmul(pops[:], attT[:, 128 * t:128 * (t + 1)],
                                         outp_t[:, 512 * nh:512 * (nh + 1)],
                                         start=True, stop=True)
                        nc.vector.tensor_copy(po[:, 512 * nh:512 * (nh + 1)], pops[:])
                    nc.sync.dma_start(out=rs1_in[128 * t:128 * (t + 1), :], in_=po[:])
                nc.gpsimd.collective_compute(
                    "ReduceScatter", mybir.AluOpType.add, replica_groups=RG,
                    ins=[rs1_in.ap().opt()], outs=[rs1_out.ap().opt()])

                nmT_bf = [p3.tile([128, TPC], BF16, tag=f"nmbf{k}", name=f"nmbf{k}")
                          for k in range(KK)]
                for ts in range(4):
                    ao = p3.tile([128, D], F32, tag="ao")
                    nc.sync.dma_start(out=ao[:],
                                      in_=rs1_out[128 * ts:128 * (ts + 1), :])
                    hs = p3.tile([128, D], F32, tag="hs")
                    nc.sync.dma_start(out=hs[:], in_=h_sl[128 * ts:128 * (ts + 1), :])
                    nc.vector.tensor_add(ao[:], ao[:], hs[:])
                    # LN2
                    nst = p3.tile([128, 2, 6], F32, tag="nst3")
                    aog = ao[:].rearrange("p (g d) -> p g d", g=2)
                    for g in range(2):
                        nc.vector.bn_stats(out=nst[:, g, :], in_=aog[:, g, :])
                    mv = p3.tile([128, 2], F32, tag="mv3")
                    nc.vector.bn_aggr(out=mv[:], in_=nst[:])
                    sd = p3.tile([128, 1], F32, tag="sd3")
                    nc.scalar.activation(sd[:], mv[:, 1:2],
                                         mybir.ActivationFunctionType.Sqrt,
                                         bias=eps_t[:])
                    rstd = p3.tile([128, 1], F32, tag="rstd3")
                    nc.vector.reciprocal(rstd[:], sd[:])
                    nm_t = p3.tile([128, D], F32, tag="nm")
                    nc.vector.tensor_scalar(out=nm_t[:], in0=ao[:],
                                            scalar1=mv[:, 0:1], scalar2=rstd[:],
                                            op0=mybir.AluOpType.subtract,
                                            op1=mybir.AluOpType.mult)
                    nc.vector.tensor_mul(nm_t[:], nm_t[:], ln2w_bc[:])
                    nc.vector.tensor_add(nm_t[:], nm_t[:], ln2b_bc[:])
                    if debug:
                        nc.sync.dma_start(out=dbg['nm'][128 * ts:128 * (ts + 1), :],
                                          in_=nm_t[:])
                    # transpose nm tile; f32 slices for router, bf16 for AG
                    nmf = p3.tile([128, KK, 128], F32, tag="nmf")
                    for k in range(KK):
                        ntp = p3ps.tile([128, 128], F32, tag="ntp")
                        nc.tensor.transpose(ntp[:], nm_t[:, 128 * k:128 * (k + 1)],
                                            ident[:])
                        nc.vector.tensor_copy(nmf[:, k, :], ntp[:])
                        nc.vector.tensor_copy(
                            nmT_bf[k][:, 128 * ts:128 * (ts + 1)], ntp[:])
                    # router logits [128 tok, 8]
                    lgps = p3ps.tile([128, E], F32, tag="lgps")
                    for k in range(KK):
                        nc.tensor.matmul(lgps[:], nmf[:, k, :], gw_t[:, k, :],
                                         start=(k == 0), stop=(k == KK - 1))
                    lg = p3.tile([128, E], F32, tag="lg")
                    nc.vector.tensor_add(lg[:], lgps[:], gb_bc[:])
                    nc.sync.dma_start(out=logits_sl[128 * ts:128 * (ts + 1), :],
                                      in_=lg[:])
                    # top-2 weights
                    m1 = p3.tile([128, 1], F32, tag="m1")
                    nc.vector.tensor_reduce(out=m1[:], in_=lg[:],
                                            axis=mybir.AxisListType.X,
                                            op=mybir.AluOpType.max)
                    eq = p3.tile([128, E], F32, tag="eq")
                    nc.vector.tensor_scalar(out=eq[:], in0=lg[:], scalar1=m1[:],
                                            scalar2=None,
                                            op0=mybir.AluOpType.is_equal)
                    lmsk = p3.tile([128, E], F32, tag="lmsk")
                    nc.vector.tensor_scalar(out=lmsk[:], in0=eq[:], scalar1=-1e30,
                                            scalar2=None, op0=mybir.AluOpType.mult)
                    nc.vector.tensor_add(lmsk[:], lmsk[:], lg[:])
                    m2 = p3.tile([128, 1], F32, tag="m2")
                    nc.vector.tensor_reduce(out=m2[:], in_=lmsk[:],
                                            axis=mybir.AxisListType.X,
                                            op=mybir.AluOpType.max)
                    nm1 = p3.tile([128, 1], F32, tag="nm1")
                    nc.vector.tensor_scalar(out=nm1[:], in0=m1[:], scalar1=-1.0,
                                            scalar2=None, op0=mybir.AluOpType.mult)
                    ev = p3.tile([128, E], F32, tag="ev")
                    nc.scalar.activation(ev[:], lg[:],
                                         mybir.ActivationFunctionType.Exp,
                                         bias=nm1[:])
                    dt_ = p3.tile([128, 1], F32, tag="dt")
                    nc.scalar.activation(dt_[:], m2[:],
                                         mybir.ActivationFunctionType.Exp,
                                         bias=nm1[:])
                    nc.vector.tensor_scalar(out=dt_[:], in0=dt_[:], scalar1=1.0,
                                            scalar2=None, op0=mybir.AluOpType.add)
                    inv = p3.tile([128, 1], F32, tag="inv")
                    nc.vector.reciprocal(inv[:], dt_[:])
                    ge = p3.tile([128, E], F32, tag="ge")
                    nc.vector.tensor_scalar(out=ge[:], in0=lg[:], scalar1=m2[:],
                                            scalar2=None, op0=mybir.AluOpType.is_ge)
                    wt = p3.tile([128, E], F32, tag="wt")
                    nc.vector.tensor_mul(wt[:], ge[:], ev[:])
                    nc.vector.tensor_scalar_mul(wt[:], wt[:], inv[:])
                    nc.sync.dma_start(out=w_in[128 * ts:128 * (ts + 1), :], in_=wt[:])
                for k in range(KK):
                    nc.sync.dma_start(out=nmT_in[128 * k:128 * (k + 1), :],
                                      in_=nmT_bf[k][:])
                nc.gpsimd.collective_compute(
                    "AllGather", mybir.AluOpType.bypass, replica_groups=RG,
                    ins=[nmT_in.ap().opt()], outs=[nmT_full.ap().opt()])
                nc.gpsimd.collective_compute(
                    "AllGather", mybir.AluOpType.bypass, replica_groups=RG,
                    ins=[w_in.ap().opt()], outs=[w_full.ap().opt()])

            persist_cm.__exit__(None, None, None)

            # ================= Phase 4: expert MLP (bf16) =================
            with tc.tile_pool(name="p4c", bufs=1) as p4c, \
                 tc.tile_pool(name="p4", bufs=2) as p4, \
                 tc.tile_pool(name="p4w", bufs=3) as p4w, \
                 tc.tile_pool(name="p4ps", bufs=2, space="PSUM") as p4ps:
                w3_t = p4c.tile([128, KM, D], BF16)
                nc.sync.dma_start(
                    out=w3_t[:], in_=w3e.ap().rearrange("(m p) d -> p m d", p=128))
                # per-token weight of this core's expert: sum(w_full * onehot)
                we_t = p4c.tile([128, NT], F32)
                for j in range(NT):
                    wf = p4.tile([128, E], F32, tag="wf")
                    nc.sync.dma_start(out=wf[:],
                                      in_=w_full[128 * j:128 * (j + 1), :])
                    nc.vector.tensor_mul(wf[:], wf[:], oh_bc[:])
                    nc.vector.tensor_reduce(out=we_t[:, j:j + 1], in_=wf[:],
                                            axis=mybir.AxisListType.X,
                                            op=mybir.AluOpType.add)
                if debug:
                    wdb = p4.tile([128, E], F32, tag="wdb")
                    for ts in range(4):
                        nc.sync.dma_start(out=wdb[:],
                                          in_=w_full[128 * ts:128 * (ts + 1), :])
                        nc.sync.dma_start(out=dbg['w'][128 * ts:128 * (ts + 1), :],
                                          in_=wdb[:])

                for tb in range(NC):
                    nmtb = [p4.tile([128, TPC], BF16, tag=f"nmtb{k}", name=f"nmtb{k}")
                            for k in range(KK)]
                    for k in range(KK):
                        nc.sync.dma_start(
                            out=nmtb[k][:],
                            in_=nmT_full[D * tb + 128 * k:D * tb + 128 * (k + 1), :])
                    h1T = p4.tile([128, KM, TPC], BF16, tag="h1T", bufs=1)
                    for m in range(KM):
                        w1t = p4w.tile([128, KK, 128], BF16, tag="w1t")
                        nc.sync.dma_start(
                            out=w1t[:],
                            in_=bass.AP(tensor=w1e, offset=128 * m,
                                        ap=[[HID, 128], [128 * HID, KK], [1, 128]]))
                        w2t = p4w.tile([128, KK, 128], BF16, tag="w2t")
                        nc.sync.dma_start(
                            out=w2t[:],
                            in_=bass.AP(tensor=w2e, offset=128 * m,
                                        ap=[[HID, 128], [128 * HID, KK], [1, 128]]))
                        y1 = p4ps.tile([128, TPC], F32, tag="y1")
                        for k in range(KK):
                            nc.tensor.matmul(y1[:], w1t[:, k, :], nmtb[k][:],
                                             start=(k == 0), stop=(k == KK - 1))
                        y2 = p4ps.tile([128, TPC], F32, tag="y2")
                        for k in range(KK):
                            nc.tensor.matmul(y2[:], w2t[:, k, :], nmtb[k][:],
                                             start=(k == 0), stop=(k == KK - 1))
                        sil = p4.tile([128, TPC], F32, tag="sil")
                        nc.scalar.activation(sil[:], y1[:],
                                             mybir.ActivationFunctionType.Silu,
                                             bias=b1_t[:, m:m + 1])
                        # h1 = (y2 + b2) * silu  (one fused DVE op)
                        nc.vector.scalar_tensor_tensor(
                            out=h1T[:, m, :], in0=y2[:], scalar=b2_t[:, m:m + 1],
                            in1=sil[:], op0=mybir.AluOpType.add,
                            op1=mybir.AluOpType.mult)
                    for ts in range(4):
                        wcol = we_t[:, 4 * tb + ts:4 * tb + ts + 1]
                        for nh in range(2):
                            ey = p4ps.tile([128, 512], F32, tag="ey")
                            for m in range(KM):
                                nc.tensor.matmul(
                                    ey[:], h1T[:, m, 128 * ts:128 * (ts + 1)],
                                    w3_t[:, m, 512 * nh:512 * (nh + 1)],
                                    start=(m == 0), stop=(m == KM - 1))
                            eyb = p4.tile([128, 512], F32, tag="eyb")
                            nc.vector.tensor_add(eyb[:], ey[:],
                                                 b3_bc[:, 512 * nh:512 * (nh + 1)])
                            nc.vector.tensor_scalar_mul(eyb[:], eyb[:], wcol)
                            nc.sync.dma_start(
                                out=rs2_in[TPC * tb + 128 * ts:
                                           TPC * tb + 128 * (ts + 1),
                                           512 * nh:512 * (nh + 1)],
                                in_=eyb[:])
                if debug:
                    mdb = p4.tile([128, D], F32, tag="mdb")
                    for t in range(NT):
                        nc.sync.dma_start(out=mdb[:],
                                          in_=rs2_in[128 * t:128 * (t + 1), :])
                        nc.sync.dma_start(out=dbg['moein'][128 * t:128 * (t + 1), :],
                                          in_=mdb[:])
                nc.gpsimd.collective_compute(
                    "ReduceScatter", mybir.AluOpType.add, replica_groups=RG,
                    ins=[rs2_in.ap().opt()], outs=[rs2_out.ap().opt()])
                fin = p4.tile([128, D], F32, tag="fin")
                for ts in range(4):
                    nc.sync.dma_start(out=fin[:],
                                      in_=rs2_out[128 * ts:128 * (ts + 1), :])
                    nc.sync.dma_start(out=out_sl[128 * ts:128 * (ts + 1), :],
                                      in_=fin[:])

    nc.finalize()
    _CACHE[key] = nc
    return nc


def _host_prep(inputs):
    """Host-side input prep: embedding gather, weight slicing/permutation."""
    x = np.asarray(inputs['x'])
    emb = np.asarray(inputs['emb'], dtype=np.float32)
    h = emb[x.reshape(-1)]                      # [NTOK, D] f32
    wq = np.asarray(inputs['q_proj'], np.float32)
    wk = np.asarray(inputs['k_proj'], np.float32)
    wv = np.asarray(inputs['v_proj'], np.float32)
    outp = np.asarray(inputs['out_proj'], np.float32)
    w1 = np.asarray(inputs['w1'], np.float32)
    w2 = np.asarray(inputs['w2'], np.float32)
    w3 = np.asarray(inputs['w3'], np.float32)

    half = DH // 2
    inv_freq = 1.0 / (10000.0 ** (np.arange(half, dtype=np.float32) / half))
    ang = np.arange(N, dtype=np.float32)[:, None] * inv_freq    # [N, 32]
    cos = np.cos(ang).astype(np.float32)
    sin = np.sin(ang).astype(np.float32)
    cos_full = np.tile(cos, (B, 1))             # [NTOK, 32]
    sin_full = np.tile(sin, (B, 1))

    perm = np.concatenate([np.arange(0, DH, 2), np.arange(1, DH, 2)])  # de-interleave

    in_maps = []
    for c in range(NC):
        heads = [2 * c, 2 * c + 1]
        qcols = np.concatenate([wq[:, DH * th + perm] for th in heads], axis=1)
        kcols = np.concatenate([wk[:, DH * th + perm] for th in heads], axis=1)
        vcols = np.concatenate([wv[:, DH * th:DH * (th + 1)] for th in heads], axis=1)
        wqkv = np.concatenate([qcols, kcols, vcols], axis=1)      # [D, 384]
        onehot = np.zeros(E, np.float32)
        onehot[c] = 1.0
        m = {
            'h': h,
            'h_sl': h[TPC * c:TPC * (c + 1)],
            'wqkv': np.ascontiguousarray(wqkv),
            'outp': np.ascontiguousarray(outp[128 * c:128 * (c + 1), :]),
            'ln1w': np.asarray(inputs['ln1_w'], np.float32),
            'ln1b': np.asarray(inputs['ln1_b'], np.float32),
            'ln2w': np.asarray(inputs['ln2_w'], np.float32),
            'ln2b': np.asarray(inputs['ln2_b'], np.float32),
            'gw': np.asarray(inputs['gate_w'], np.float32),
            'gb': np.asarray(inputs['gate_b'], np.float32),
            'w1e': w1[c].astype(ml_dtypes.bfloat16),
            'w2e': w2[c].astype(ml_dtypes.bfloat16),
            'w3e': w3[c].astype(ml_dtypes.bfloat16),
            'b1e': np.asarray(inputs['b1'], np.float32)[c],
            'b2e': np.asarray(inputs['b2'], np.float32)[c],
            'b3e': np.asarray(inputs['b3'], np.float32)[c],
            'cos': cos_full,
            'sin': sin_full,
            'onehot': onehot,
        }
        in_maps.append(m)
    return in_maps


def _aux_from_logits(logits):
    """Replicate the reference aux-loss math in numpy f32."""
    lg = logits.astype(np.float32)                      # [NTOK, E]
    m = lg.max(axis=1, keepdims=True)
    p = np.exp(lg - m)
    p /= p.sum(axis=1, keepdims=True)
    importance = p.mean(axis=0)
    i1 = lg.argmax(axis=1)
    lg2 = lg.copy()
    lg2[np.arange(lg.shape[0]), i1] = -np.inf
    i2 = lg2.argmax(axis=1)
    load = (np.bincount(i1, minlength=E) + np.bincount(i2, minlength=E)).astype(
        np.float32) / (NTOK * 2)
    return np.float32((importance * load).sum() * E)


def kernel(**inputs):
    nc = _build(debug=False)
    in_maps = _host_prep(inputs)
    r = run_bass_kernel_spmd(nc, in_maps, core_ids=list(range(NC)))
    res = r.results
    out = np.concatenate([res[c]['out_sl'] for c in range(NC)], axis=0)
    out = out.reshape(B, N, D)
    logits = np.concatenate([res[c]['logits_sl'] for c in range(NC)], axis=0)
    aux = _aux_from_logits(logits)
    return out, np.asarray(aux, np.float32)


# revision 6
# speedup vs baseline: 1.0061x; 1.0061x over previous
"""Trainium2 Bass kernel for nn_Decoder_74234214744419 (moe_routing).

Decoder layer: embedding lookup -> LN1 -> RoPE causal attention ->
out_proj+residual -> LN2 -> router (top-2 of 8) -> 8 dense SwiGLU experts ->
weighted combine; plus load-balance aux loss.

Distribution over 8 NeuronCores:
 - attention head-parallel (16 heads -> 2 per core); QKV/attention math in
   fp32 (the top-2 routing is a discrete function of the logits, which sit
   downstream of attention - bf16 noise there flips expert selections vs the
   reference, so this path stays fp32)
 - out_proj computed as per-core partials over its own heads, combined with
   an on-device ReduceScatter -> each core owns a 512-token slice
 - LN2 + router token-sharded; router weights AllGather'd
 - nm^T (bf16) AllGather'd; experts run expert-parallel (core c = expert c),
   MoE matmuls in bf16
 - weighted expert outputs combined with an on-device ReduceScatter
 - host: embedding gather (pure data movement), aux loss from
   device-computed logits, final slice concat

RoPE trick: the interleaved (even,odd) pair rotation is turned into a
contiguous-half rotation by permuting the q/k weight COLUMNS on the host
(per-head de-interleave). Scores are invariant to any per-head permutation
applied to both q and k.
"""
import sys
if '/opt/trn_rl_repo' not in sys.path:
    sys.path.insert(0, '/opt/trn_rl_repo')

import numpy as np
import ml_dtypes

import concourse.bass as bass
import concourse.tile as tile
from concourse import bacc, mybir
from concourse.bass_utils import run_bass_kernel_spmd
from concourse.masks import make_identity, make_causal_mask

F32 = mybir.dt.float32
BF16 = mybir.dt.bfloat16

B, N, D, H, DH, E, HID, V = 2, 2048, 1024, 16, 64, 8, 4096, 32000
NTOK = B * N            # 4096
NC = 8                  # cores
TPC = NTOK // NC        # 512 tokens per core
NT = NTOK // 128        # 32 token tiles
KK = D // 128           # 8 contraction tiles over D
KM = HID // 128         # 32 contraction tiles over HID
NEG = -400.0            # additive causal mask; Exp applies scale=0.125 -> exp(-50)

_CACHE = {}


def _build(debug=False):
    key = ('nc', debug)
    if key in _CACHE:
        return _CACHE[key]
    nc = bacc.Bacc(num_devices=NC)

    # ---- I/O ----
    h_in = nc.dram_tensor("h", [NTOK, D], F32, kind="ExternalInput")
    h_sl = nc.dram_tensor("h_sl", [TPC, D], F32, kind="ExternalInput")
    wqkv = nc.dram_tensor("wqkv", [D, 384], F32, kind="ExternalInput")
    outp = nc.dram_tensor("outp", [128, D], F32, kind="ExternalInput")
    ln1w = nc.dram_tensor("ln1w", [D], F32, kind="ExternalInput")
    ln1b = nc.dram_tensor("ln1b", [D], F32, kind="ExternalInput")
    ln2w = nc.dram_tensor("ln2w", [D], F32, kind="ExternalInput")
    ln2b = nc.dram_tensor("ln2b", [D], F32, kind="ExternalInput")
    gw = nc.dram_tensor("gw", [D, E], F32, kind="ExternalInput")
    gb = nc.dram_tensor("gb", [E], F32, kind="ExternalInput")
    w1e = nc.dram_tensor("w1e", [D, HID], BF16, kind="ExternalInput")
    w2e = nc.dram_tensor("w2e", [D, HID], BF16, kind="ExternalInput")
    w3e = nc.dram_tensor("w3e", [HID, D], BF16, kind="ExternalInput")
    b1e = nc.dram_tensor("b1e", [HID], F32, kind="ExternalInput")
    b2e = nc.dram_tensor("b2e", [HID], F32, kind="ExternalInput")
    b3e = nc.dram_tensor("b3e", [D], F32, kind="ExternalInput")
    cos_in = nc.dram_tensor("cos", [NTOK, 32], F32, kind="ExternalInput")
    sin_in = nc.dram_tensor("sin", [NTOK, 32], F32, kind="ExternalInput")
    onehot = nc.dram_tensor("onehot", [E], F32, kind="ExternalInput")

    out_sl = nc.dram_tensor("out_sl", [TPC, D], F32, kind="ExternalOutput")
    logits_sl = nc.dram_tensor("logits_sl", [TPC, E], F32, kind="ExternalOutput")
    dbg = {}
    if debug:
        dbg['qkT0'] = nc.dram_tensor("dbg_qkT0", [128, NTOK], F32, kind="ExternalOutput")
        dbg['qkT1'] = nc.dram_tensor("dbg_qkT1", [128, NTOK], F32, kind="ExternalOutput")
        dbg['attT'] = nc.dram_tensor("dbg_attT", [128, NTOK], F32, kind="ExternalOutput")
        dbg['nm'] = nc.dram_tensor("dbg_nm", [TPC, D], F32, kind="ExternalOutput")
        dbg['w'] = nc.dram_tensor("dbg_w", [TPC, E], F32, kind="ExternalOutput")
        dbg['moein'] = nc.dram_tensor("dbg_moein", [NTOK, D], F32, kind="ExternalOutput")

    # ---- internal DRAM (collectives) ----
    rs1_in = nc.dram_tensor("rs1_in", [NTOK, D], F32)
    rs1_out = nc.dram_tensor("rs1_out", [TPC, D], F32)
    nmT_in = nc.dram_tensor("nmT_in", [D, TPC], BF16)
    nmT_full = nc.dram_tensor("nmT_full", [NC * D, TPC], BF16, addr_space="Shared")
    w_in = nc.dram_tensor("w_in", [TPC, E], F32)
    w_full = nc.dram_tensor("w_full", [NTOK, E], F32, addr_space="Shared")
    rs2_in = nc.dram_tensor("rs2_in", [NTOK, D], F32)
    rs2_out = nc.dram_tensor("rs2_out", [TPC, D], F32)
    RG = [list(range(NC))]

    with tile.TileContext(nc) as tc:
        with tc.tile_pool(name="const", bufs=1) as const:
            ident = const.tile([128, 128], F32)
            make_identity(nc, ident)
            cmask = const.tile([128, 128], F32)
            make_causal_mask(nc, cmask, mask_val=NEG)
            eps_t = const.tile([128, 1], F32)
            nc.vector.memset(eps_t, 1e-5)

            def bcast_load(vec, n):
                t = const.tile([128, n], F32, tag=f"bc_{vec.name}")
                src = bass.AP(tensor=vec, offset=0, ap=[[0, 128], [1, n]])
                nc.sync.dma_start(out=t[:], in_=src)
                return t

            ln1w_bc = bcast_load(ln1w, D)
            ln1b_bc = bcast_load(ln1b, D)
            ln2w_bc = bcast_load(ln2w, D)
            ln2b_bc = bcast_load(ln2b, D)
            b3_bc = bcast_load(b3e, D)
            gb_bc = bcast_load(gb, E)
            oh_bc = bcast_load(onehot, E)

            wqkv_t = const.tile([128, KK, 384], F32)
            nc.sync.dma_start(out=wqkv_t[:],
                              in_=wqkv.ap().rearrange("(k p) c -> p k c", p=128))
            outp_t = const.tile([128, D], F32)
            nc.sync.dma_start(out=outp_t[:], in_=outp[:, :])
            gw_t = const.tile([128, KK, E], F32)
            nc.sync.dma_start(out=gw_t[:],
                              in_=gw.ap().rearrange("(k p) e -> p k e", p=128))
            # b1/b2 partition-major: element m*128+p -> [p, m]
            b1_t = const.tile([128, KM], F32)
            nc.sync.dma_start(out=b1_t[:],
                              in_=bass.AP(tensor=b1e, offset=0, ap=[[1, 128], [128, KM]]))
            b2_t = const.tile([128, KM], F32)
            nc.sync.dma_start(out=b2_t[:],
                              in_=bass.AP(tensor=b2e, offset=0, ap=[[1, 128], [128, KM]]))

            # persistent activation tensors (live through phases 1-3 only)
            persist_cm = tc.tile_pool(name="persist", bufs=1)
            persist = persist_cm.__enter__()
            qkT0 = persist.tile([128, NTOK], F32, tag="qkT0")  # q^T (2 heads)
            qkT1 = persist.tile([128, NTOK], F32, tag="qkT1")  # k^T
            v_t = persist.tile([128, NT, 128], F32, tag="v")   # v token-major
            attT = persist.tile([128, NTOK], F32, tag="attT")  # att^T (2 heads)

            # ================= Phase 1+2: LN1 + QKV + RoPE =================
            with tc.tile_pool(name="p12", bufs=3) as p12, \
                 tc.tile_pool(name="p12ps", bufs=2, space="PSUM") as p12ps:
                for t in range(NT):
                    h_t = p12.tile([128, D], F32, tag="h")
                    nc.sync.dma_start(out=h_t[:], in_=h_in[128 * t:128 * (t + 1), :])
                    # LN1
                    nst = p12.tile([128, 2, 6], F32, tag="nst")
                    hg = h_t[:].rearrange("p (g d) -> p g d", g=2)
                    for g in range(2):
                        nc.vector.bn_stats(out=nst[:, g, :], in_=hg[:, g, :])
                    mv = p12.tile([128, 2], F32, tag="mv")
                    nc.vector.bn_aggr(out=mv[:], in_=nst[:])
                    sd = p12.tile([128, 1], F32, tag="sd")
                    nc.scalar.activation(sd[:], mv[:, 1:2],
                                         mybir.ActivationFunctionType.Sqrt,
                                         bias=eps_t[:])
                    rstd = p12.tile([128, 1], F32, tag="rstd")
                    nc.vector.reciprocal(rstd[:], sd[:])
                    pn_t = p12.tile([128, D], F32, tag="pn")
                    nc.vector.tensor_scalar(out=pn_t[:], in0=h_t[:],
                                            scalar1=mv[:, 0:1], scalar2=rstd[:],
                                            op0=mybir.AluOpType.subtract,
                                            op1=mybir.AluOpType.mult)
                    nc.vector.tensor_mul(pn_t[:], pn_t[:], ln1w_bc[:])
                    nc.vector.tensor_add(pn_t[:], pn_t[:], ln1b_bc[:])
                    # transpose pn tile -> 8 [128,128] slices (D on partitions)
                    trs = p12.tile([128, KK, 128], F32, tag="trs")
                    for k in range(KK):
                        trp = p12ps.tile([128, 128], F32, tag="trp")
                        nc.tensor.transpose(trp[:], pn_t[:, 128 * k:128 * (k + 1)],
                                            ident[:])
                        nc.vector.tensor_copy(trs[:, k, :], trp[:])
                    # qkv = pn @ wqkv : [128 tok, 384]
                    qkv_ps = p12ps.tile([128, 384], F32, tag="qkv")
                    for k in range(KK):
                        nc.tensor.matmul(qkv_ps[:], trs[:, k, :], wqkv_t[:, k, :],
                                         start=(k == 0), stop=(k == KK - 1))
                    # rope on q/k blocks (per-head [e(32) o(32)] layout)
                    cs = p12.tile([128, 32], F32, tag="cs")
                    nc.sync.dma_start(out=cs[:], in_=cos_in[128 * t:128 * (t + 1), :])
                    sn = p12.tile([128, 32], F32, tag="sn")
                    nc.sync.dma_start(out=sn[:], in_=sin_in[128 * t:128 * (t + 1), :])
                    rot = p12.tile([128, 256], F32, tag="rot")
                    t1 = p12.tile([128, 32], F32, tag="t1")
                    t2 = p12.tile([128, 32], F32, tag="t2")
                    for qk in range(2):
                        for th in range(2):
                            base = 128 * qk + 64 * th
                            qe = qkv_ps[:, base:base + 32]
                            qo = qkv_ps[:, base + 32:base + 64]
                            re = rot[:, base:base + 32]
                            ro = rot[:, base + 32:base + 64]
                            nc.vector.tensor_mul(t1[:], qe, cs[:])
                            nc.vector.tensor_mul(t2[:], qo, sn[:])
                            nc.vector.tensor_sub(re, t1[:], t2[:])
                            nc.vector.tensor_mul(t1[:], qe, sn[:])
                            nc.vector.tensor_mul(t2[:], qo, cs[:])
                            nc.vector.tensor_add(ro, t1[:], t2[:])
                    # v
                    nc.vector.tensor_copy(v_t[:, t, :], qkv_ps[:, 256:384])
                    # transpose rot -> qkT0/qkT1 columns
                    for blk, dst in ((0, qkT0), (1, qkT1)):
                        trp2 = p12ps.tile([128, 128], F32, tag="trp")
                        nc.tensor.transpose(trp2[:],
                                            rot[:, 128 * blk:128 * (blk + 1)],
                                            ident[:])
                        nc.vector.tensor_copy(dst[:, 128 * t:128 * (t + 1)], trp2[:])

            if debug:
                nc.sync.dma_start(out=dbg['qkT0'][:, :], in_=qkT0[:])
                nc.sync.dma_start(out=dbg['qkT1'][:, :], in_=qkT1[:])

            # ================= Phase A: causal attention =================
            # software-pipelined across the two local heads: both heads'
            # score/softmax chains are issued before either head's
            # transpose+PV block, so the PE always has matmul work while
            # ACT/DVE run the other head's exp/normalize (keeps HAM warm).
            with tc.tile_pool(name="pat", bufs=2) as pat, \
                 tc.tile_pool(name="patp", bufs=2, space="PSUM") as patp:
                for b in range(B):
                    for qt in range(16):
                        qoff = 2048 * b + 128 * qt
                        nkey = 128 * (qt + 1)
                        prows = []
                        for hh in range(2):
                            hrow = 64 * hh
                            prow = pat.tile([128, 2048], F32, tag="prow",
                                            bufs=4, name=f"prow{hh}")
                            ltot = pat.tile([128, 1], F32, tag="ltot", bufs=4,
                                            name=f"ltot{hh}")
                            koff = 0
                            ci = 0
                            while koff < nkey:
                                csz = min(512, nkey - koff)
                                sps = patp.tile([128, 512], F32, tag="sps",
                                                bufs=3, name="sps")
                                nc.tensor.matmul(
                                    sps[:, :csz],
                                    qkT0[hrow:hrow + 64, qoff:qoff + 128],
                                    qkT1[hrow:hrow + 64,
                                         2048 * b + koff:2048 * b + koff + csz],
                                    start=True, stop=True)
                                if koff + csz == nkey:  # diagonal block
                                    nc.vector.tensor_add(sps[:, csz - 128:csz],
                                                         sps[:, csz - 128:csz],
                                                         cmask[:])
                                lsum = pat.tile([128, 1], F32, tag="lsum",
                                                bufs=4, name="lsum")
                                nc.scalar.activation(prow[:, koff:koff + csz],
                                                     sps[:, :csz],
                                                     mybir.ActivationFunctionType.Exp,
                                                     scale=0.125, accum_out=lsum[:])
                                if ci == 0:
                                    nc.vector.tensor_copy(ltot[:], lsum[:])
                                else:
                                    nc.vector.tensor_add(ltot[:], ltot[:], lsum[:])
                                koff += csz
                                ci += 1
                            linv = pat.tile([128, 1], F32, tag="linv", bufs=4,
                                            name=f"linv{hh}")
                            nc.vector.reciprocal(linv[:], ltot[:])
                            nc.vector.tensor_scalar_mul(prow[:, :nkey],
                                                        prow[:, :nkey], linv[:])
                            prows.append(prow)
                        for hh in range(2):
                            hrow = 64 * hh
                            prow = prows[hh]
                            attps = patp.tile([64, 128], F32, tag="attps",
                                              bufs=2, name="attps")
                            for j in range(qt + 1):
                                ptp = patp.tile([128, 128], F32, tag="ptp",
                                                bufs=2, name="ptp")
                                nc.tensor.transpose(
                                    ptp[:], prow[:, 128 * j:128 * (j + 1)], ident[:])
                                pts = pat.tile([128, 128], F32, tag="pts",
                                               bufs=3, name="pts")
                                nc.vector.tensor_copy(pts[:], ptp[:])
                                nc.tensor.matmul(
                                    attps[:], v_t[:, 16 * b + j, hrow:hrow + 64],
                                    pts[:], start=(j == 0), stop=(j == qt),
                                    skip_group_check=True)
                            nc.vector.tensor_copy(attT[hrow:hrow + 64,
                                                       qoff:qoff + 128], attps[:])

            if debug:
                nc.sync.dma_start(out=dbg['attT'][:, :], in_=attT[:])

            # ============ Phase 3: out_proj partials + RS + LN2 + router ============
            with tc.tile_pool(name="p3", bufs=2) as p3, \
                 tc.tile_pool(name="p3ps", bufs=2, space="PSUM") as p3ps:
                for t in range(NT):
                    po = p3.tile([128, D], F32, tag="po")
                    for nh in range(2):
                        pops = p3ps.tile([128, 512], F32, tag="pops")
                        nc.tensor.matmul(pops[:], attT[:, 128 * t:128 * (t + 1)],
                                         outp_t[:, 512 * nh:512 * (nh + 1)],
                                         start=True, stop=True)
                        nc.vector.tensor_copy(po[:, 512 * nh:512 * (nh + 1)], pops[:])
                    nc.sync.dma_start(out=rs1_in[128 * t:128 * (t + 1), :], in_=po[:])
                nc.gpsimd.collective_compute(
                    "ReduceScatter", mybir.AluOpType.add, replica_groups=RG,
                    ins=[rs1_in.ap().opt()], outs=[rs1_out.ap().opt()])

                nmT_bf = [p3.tile([128, TPC], BF16, tag=f"nmbf{k}", name=f"nmbf{k}")
                          for k in range(KK)]
                for ts in range(4):
                    ao = p3.tile([128, D], F32, tag="ao")
                    nc.sync.dma_start(out=ao[:],
                                      in_=rs1_out[128 * ts:128 * (ts + 1), :])
                    hs = p3.tile([128, D], F32, tag="hs")
                    nc.sync.dma_start(out=hs[:], in_=h_sl[128 * ts:128 * (ts + 1), :])
                    nc.vector.tensor_add(ao[:], ao[:], hs[:])
                    # LN2
                    nst = p3.tile([128, 2, 6], F32, tag="nst3")
                    aog = ao[:].rearrange("p (g d) -> p g d", g=2)
                    for g in range(2):
                        nc.vector.bn_stats(out=nst[:, g, :], in_=aog[:, g, :])
                    mv = p3.tile([128, 2], F32, tag="mv3")
                    nc.vector.bn_aggr(out=mv[:], in_=nst[:])
                    sd = p3.tile([128, 1], F32, tag="sd3")
                    nc.scalar.activation(sd[:], mv[:, 1:2],
                                         mybir.ActivationFunctionType.Sqrt,
                                         bias=eps_t[:])
                    rstd = p3.tile([128, 1], F32, tag="rstd3")
                    nc.vector.reciprocal(rstd[:], sd[:])
                    nm_t = p3.tile([128, D], F32, tag="nm")
                    nc.vector.tensor_scalar(out=nm_t[:], in0=ao[:],
                                            scalar1=mv[:, 0:1], scalar2=rstd[:],
                                            op0=mybir.AluOpType.subtract,
                                            op1=mybir.AluOpType.mult)
                    nc.vector.tensor_mul(nm_t[:], nm_t[:], ln2w_bc[:])
                    nc.vector.tensor_add(nm_t[:], nm_t[:], ln2b_bc[:])
                    if debug:
                        nc.sync.dma_start(out=dbg['nm'][128 * ts:128 * (ts + 1), :],
                                          in_=nm_t[:])
                    # transpose nm tile; f32 slices for router, bf16 for AG
                    nmf = p3.tile([128, KK, 128], F32, tag="nmf")
                    for k in range(KK):
                        ntp = p3ps.tile([128, 128], F32, tag="ntp")
                        nc.tensor.transpose(ntp[:], nm_t[:, 128 * k:128 * (k + 1)],
                                            ident[:])
                        nc.vector.tensor_copy(nmf[:, k, :], ntp[:])
                        nc.vector.tensor_copy(
                            nmT_bf[k][:, 128 * ts:128 * (ts + 1)], ntp[:])
                    # router logits [128 tok, 8]
                    lgps = p3ps.tile([128, E], F32, tag="lgps")
                    for k in range(KK):
                        nc.tensor.matmul(lgps[:], nmf[:, k, :], gw_t[:, k, :],
                                         start=(k == 0), stop=(k == KK - 1))
                    lg = p3.tile([128, E], F32, tag="lg")
                    nc.vector.tensor_add(lg[:], lgps[:], gb_bc[:])
                    nc.sync.dma_start(out=logits_sl[128 * ts:128 * (ts + 1), :],
                                      in_=lg[:])
                    # top-2 weights
                    m1 = p3.tile([128, 1], F32, tag="m1")
                    nc.vector.tensor_reduce(out=m1[:], in_=lg[:],
                                            axis=mybir.AxisListType.X,
                                            op=mybir.AluOpType.max)
                    eq = p3.tile([128, E], F32, tag="eq")
                    nc.vector.tensor_scalar(out=eq[:], in0=lg[:], scalar1=m1[:],
                                            scalar2=None,
                                            op0=mybir.AluOpType.is_equal)
                    lmsk = p3.tile([128, E], F32, tag="lmsk")
                    nc.vector.tensor_scalar(out=lmsk[:], in0=eq[:], scalar1=-1e30,
                                            scalar2=None, op0=mybir.AluOpType.mult)
                    nc.vector.tensor_add(lmsk[:], lmsk[:], lg[:])
                    m2 = p3.tile([128, 1], F32, tag="m2")
                    nc.vector.tensor_reduce(out=m2[:], in_=lmsk[:],
                                            axis=mybir.AxisListType.X,
                                            op=mybir.AluOpType.max)
                    nm1 = p3.tile([128, 1], F32, tag="nm1")
                    nc.vector.tensor_scalar(out=nm1[:], in0=m1[:], scalar1=-1.0,
                                            scalar2=None, op0=mybir.AluOpType.mult)
                    ev = p3.tile([128, E], F32, tag="ev")
                    nc.scalar.activation(ev[:], lg[:],
                                         mybir.ActivationFunctionType.Exp,
                                         bias=nm1[:])
                    dt_ = p3.tile([128, 1], F32, tag="dt")
                    nc.scalar.activation(dt_[:], m2[:],
                                         mybir.ActivationFunctionType.Exp,
                                         bias=nm1[:])
                    nc.vector.tensor_scalar(out=dt_[:], in0=dt_[:], scalar1=1.0,
                                            scalar2=None, op0=mybir.AluOpType.add)
                    inv = p3.tile([128, 1], F32, tag="inv")
                    nc.vector.reciprocal(inv[:], dt_[:])
                    ge = p3.tile([128, E], F32, tag="ge")
                    nc.vector.tensor_scalar(out=ge[:], in0=lg[:], scalar1=m2[:],
                                            scalar2=None, op0=mybir.AluOpType.is_ge)
                    wt = p3.tile([128, E], F32, tag="wt")
                    nc.vector.tensor_mul(wt[:], ge[:], ev[:])
                    nc.vector.tensor_scalar_mul(wt[:], wt[:], inv[:])
                    nc.sync.dma_start(out=w_in[128 * ts:128 * (ts + 1), :], in_=wt[:])
                for k in range(KK):
                    nc.sync.dma_start(out=nmT_in[128 * k:128 * (k + 1), :],
                                      in_=nmT_bf[k][:])
                nc.gpsimd.collective_compute(
                    "AllGather", mybir.AluOpType.bypass, replica_groups=RG,
                    ins=[nmT_in.ap().opt()], outs=[nmT_full.ap().opt()])
                nc.gpsimd.collective_compute(
                    "AllGather", mybir.AluOpType.bypass, replica_groups=RG,
                    ins=[w_in.ap().opt()], outs=[w_full.ap().opt()])

            persist_cm.__exit__(None, None, None)

            # ================= Phase 4: expert MLP (bf16) =================
            with tc.tile_pool(name="p4c", bufs=1) as p4c, \
                 tc.tile_pool(name="p4", bufs=2) as p4, \
                 tc.tile_pool(name="p4w", bufs=3) as p4w, \
                 tc.tile_pool(name="p4ps", bufs=2, space="PSUM") as p4ps:
                w3_t = p4c.tile([128, KM, D], BF16)
                nc.sync.dma_start(
                    out=w3_t[:], in_=w3e.ap().rearrange("(m p) d -> p m d", p=128))
                # per-token weight of this core's expert: sum(w_full * onehot)
                we_t = p4c.tile([128, NT], F32)
                for j in range(NT):
                    wf = p4.tile([128, E], F32, tag="wf")
                    nc.sync.dma_start(out=wf[:],
                                      in_=w_full[128 * j:128 * (j + 1), :])
                    nc.vector.tensor_mul(wf[:], wf[:], oh_bc[:])
                    nc.vector.tensor_reduce(out=we_t[:, j:j + 1], in_=wf[:],
                                            axis=mybir.AxisListType.X,
                                            op=mybir.AluOpType.add)
                if debug:
                    wdb = p4.tile([128, E], F32, tag="wdb")
                    for ts in range(4):
                        nc.sync.dma_start(out=wdb[:],
                                          in_=w_full[128 * ts:128 * (ts + 1), :])
                        nc.sync.dma_start(out=dbg['w'][128 * ts:128 * (ts + 1), :],
                                          in_=wdb[:])

                for tb in range(NC):
                    nmtb = [p4.tile([128, TPC], BF16, tag=f"nmtb{k}", name=f"nmtb{k}")
                            for k in range(KK)]
                    for k in range(KK):
                        nc.sync.dma_start(
                            out=nmtb[k][:],
                            in_=nmT_full[D * tb + 128 * k:D * tb + 128 * (k + 1), :])
                    h1T = p4.tile([128, KM, TPC], BF16, tag="h1T", bufs=1)
                    for m in range(KM):
                        w1t = p4w.tile([128, KK, 128], BF16, tag="w1t")
                        nc.sync.dma_start(
                            out=w1t[:],
                            in_=bass.AP(tensor=w1e, offset=128 * m,
                                        ap=[[HID, 128], [128 * HID, KK], [1, 128]]))
                        w2t = p4w.tile([128, KK, 128], BF16, tag="w2t")
                        nc.sync.dma_start(
                            out=w2t[:],
                            in_=bass.AP(tensor=w2e, offset=128 * m,
                                        ap=[[HID, 128], [128 * HID, KK], [1, 128]]))
                        y1 = p4ps.tile([128, TPC], F32, tag="y1")
                        for k in range(KK):
                            nc.tensor.matmul(y1[:], w1t[:, k, :], nmtb[k][:],
                                             start=(k == 0), stop=(k == KK - 1))
                        y2 = p4ps.tile([128, TPC], F32, tag="y2")
                        for k in range(KK):
                            nc.tensor.matmul(y2[:], w2t[:, k, :], nmtb[k][:],
                                             start=(k == 0), stop=(k == KK - 1))
                        sil = p4.tile([128, TPC], F32, tag="sil")
                        nc.scalar.activation(sil[:], y1[:],
                                             mybir.ActivationFunctionType.Silu,
                                             bias=b1_t[:, m:m + 1])
                        # h1 = (y2 + b2) * silu  (one fused DVE op)
                        nc.vector.scalar_tensor_tensor(
                            out=h1T[:, m, :], in0=y2[:], scalar=b2_t[:, m:m + 1],
                            in1=sil[:], op0=mybir.AluOpType.add,
                            op1=mybir.AluOpType.mult)
                    for ts in range(4):
                        wcol = we_t[:, 4 * tb + ts:4 * tb + ts + 1]
                        for nh in range(2):
                            ey = p4ps.tile([128, 512], F32, tag="ey")
                            for m in range(KM):
                                nc.tensor.matmul(
                                    ey[:], h1T[:, m, 128 * ts:128 * (ts + 1)],
                                    w3_t[:, m, 512 * nh:512 * (nh + 1)],
                                    start=(m == 0), stop=(m == KM - 1))
                            eyb = p4.tile([128, 512], F32, tag="eyb")
                            nc.vector.tensor_add(eyb[:], ey[:],
                                                 b3_bc[:, 512 * nh:512 * (nh + 1)])
                            nc.vector.tensor_scalar_mul(eyb[:], eyb[:], wcol)
                            nc.sync.dma_start(
                                out=rs2_in[TPC * tb + 128 * ts:
                                           TPC * tb + 128 * (ts + 1),
                                           512 * nh:512 * (nh + 1)],
                                in_=eyb[:])
                if debug:
                    mdb = p4.tile([128, D], F32, tag="mdb")
                    for t in range(NT):
                        nc.sync.dma_start(out=mdb[:],
                                          in_=rs2_in[128 * t:128 * (t + 1), :])
                        nc.sync.dma_start(out=dbg['moein'][128 * t:128 * (t + 1), :],
                                          in_=mdb[:])
                nc.gpsimd.collective_compute(
                    "ReduceScatter", mybir.AluOpType.add, replica_groups=RG,
                    ins=[rs2_in.ap().opt()], outs=[rs2_out.ap().opt()])
                fin = p4.tile([128, D], F32, tag="fin")
                for ts in range(4):
                    nc.sync.dma_start(out=fin[:],
                                      in_=rs2_out[128 * ts:128 * (ts + 1), :])
                    nc.sync.dma_start(out=out_sl[128 * ts:128 * (ts + 1), :],
                                      in_=fin[:])

    nc.finalize()
    _CACHE[key] = nc
    return nc


def _host_prep(inputs):
    """Host-side input prep: embedding gather, weight slicing/permutation."""
    x = np.asarray(inputs['x'])
    emb = np.asarray(inputs['emb'], dtype=np.float32)
    h = emb[x.reshape(-1)]                      # [NTOK, D] f32
    wq = np.asarray(inputs['q_proj'], np.float32)
    wk = np.asarray(inputs['k_proj'], np.float32)
    wv = np.asarray(inputs['v_proj'], np.float32)
    outp = np.asarray(inputs['out_proj'], np.float32)
    w1 = np.asarray(inputs['w1'], np.float32)
    w2 = np.asarray(inputs['w2'], np.float32)
    w3 = np.asarray(inputs['w3'], np.float32)

    half = DH // 2
    inv_freq = 1.0 / (10000.0 ** (np.arange(half, dtype=np.float32) / half))
    ang = np.arange(N, dtype=np.float32)[:, None] * inv_freq    # [N, 32]
    cos = np.cos(ang).astype(np.float32)
    sin = np.sin(ang).astype(np.float32)
    cos_full = np.tile(cos, (B, 1))             # [NTOK, 32]
    sin_full = np.tile(sin, (B, 1))

    perm = np.concatenate([np.arange(0, DH, 2), np.arange(1, DH, 2)])  # de-interleave

    in_maps = []
    for c in range(NC):
        heads = [2 * c, 2 * c + 1]
        qcols = np.concatenate([wq[:, DH * th + perm] for th in heads], axis=1)
        kcols = np.concatenate([wk[:, DH * th + perm] for th in heads], axis=1)
        vcols = np.concatenate([wv[:, DH * th:DH * (th + 1)] for th in heads], axis=1)
        wqkv = np.concatenate([qcols, kcols, vcols], axis=1)      # [D, 384]
        onehot = np.zeros(E, np.float32)
        onehot[c] = 1.0
        m = {
            'h': h,
            'h_sl': h[TPC * c:TPC * (c + 1)],
            'wqkv': np.ascontiguousarray(wqkv),
            'outp': np.ascontiguousarray(outp[128 * c:128 * (c + 1), :]),
            'ln1w': np.asarray(inputs['ln1_w'], np.float32),
            'ln1b': np.asarray(inputs['ln1_b'], np.float32),
            'ln2w': np.asarray(inputs['ln2_w'], np.float32),
            'ln2b': np.asarray(inputs['ln2_b'], np.float32),
            'gw': np.asarray(inputs['gate_w'], np.float32),
            'gb': np.asarray(inputs['gate_b'], np.float32),
            'w1e': w1[c].astype(ml_dtypes.bfloat16),
            'w2e': w2[c].astype(ml_dtypes.bfloat16),
            'w3e': w3[c].astype(ml_dtypes.bfloat16),
            'b1e': np.asarray(inputs['b1'], np.float32)[c],
            'b2e': np.asarray(inputs['b2'], np.float32)[c],
            'b3e': np.asarray(inputs['b3'], np.float32)[c],
            'cos': cos_full,
            'sin': sin_full,
            'onehot': onehot,
        }
        in_maps.append(m)
    return in_maps


def _aux_from_logits(logits):
    """Replicate the reference aux-loss math in numpy f32."""
    lg = logits.astype(np.float32)                      # [NTOK, E]
    m = lg.max(axis=1, keepdims=True)
    p = np.exp(lg - m)
    p /= p.sum(axis=1, keepdims=True)
    importance = p.mean(axis=0)
    i1 = lg.argmax(axis=1)
    lg2 = lg.copy()
    lg2[np.arange(lg.shape[0]), i1] = -np.inf
    i2 = lg2.argmax(axis=1)
    load = (np.bincount(i1, minlength=E) + np.bincount(i2, minlength=E)).astype(
        np.float32) / (NTOK * 2)
    return np.float32((importance * load).sum() * E)


def kernel(**inputs):
    nc = _build(debug=False)
    in_maps = _host_prep(inputs)
    r = run_bass_kernel_spmd(nc, in_maps, core_ids=list(range(NC)))
    res = r.results
    out = np.concatenate([res[c]['out_sl'] for c in range(NC)], axis=0)
    out = out.reshape(B, N, D)
    logits = np.concatenate([res[c]['logits_sl'] for c in range(NC)], axis=0)
    aux = _aux_from_logits(logits)
    return out, np.asarray(aux, np.float32)
